# revision 20
# baseline (speedup 1.0000x reference)
"""Trainium2 Bass kernel for nn_Dynamic_deformable_DySample_restart.

Problem: 3x3 conv (30->84ch) over guidance produces per-pixel offsets +
softmax affinities for 3 iterations of a modulated deformable 3x3 conv
(bilinear sampling) with restart/confidence blending.

Strategy (8 NeuronCores, pure data parallel, one NEFF):
  - shard = (batch b, H-half) -> 8 shards of 176 output rows (+ margins).
  - Phase 1 (PE): conv as 3 accumulating matmuls (K=90 = 30ch x 3kx taps,
    kx realized as column-shifted loads of bf16 guidance), bias folded into
    PSUM evacuation; exp of the 10 softmax logits on ACT; softmax denominator
    via a block-ones matmul. Fields (dy/dx fp16, e fp16, S fp16) spilled to
    internal DRAM.
  - Phase 2 (DVE+GPSIMD+ACT): per iteration, per 64-row x 2-half band:
    feat is loaded as a row-duplicated "slab" [P, 5, 612] (each partition
    carries its +-2 halo rows in the free dim -> all 5x5 neighborhood reads
    are free-dim offsets; compute APs never shift partitions).
    Bilinear sample per tap via the 3-candidate hat identity
      val = G0 + relu(dy)*(G1-G0) + relu(-dy)*(Gm1-G0)
      G_r = f + relu(dx)*Dp(b) - relu(-dx)*Dp(b-1),  Dp = forward x-diff
    which is exact for |offset| < 1 (measured max 1.03; the single
    out-of-range sample contributes < 1e-3 absolute error).
    Updated feat roundtrips through internal DRAM between iterations.
  - Zero-padding at image borders is carried in the data (host-padded
    inputs; off-image update rows are masked to zero via om_conf/conf_ff).
"""
import os
import numpy as np
import ml_dtypes
from contextlib import ExitStack

import concourse.bacc as bacc
import concourse.bass as bass
import concourse.tile as tile
import concourse.mybir as mybir
from concourse.bass_utils import run_bass_kernel_spmd

F32 = mybir.dt.float32
F16 = mybir.dt.float16
BF16 = mybir.dt.bfloat16
ALU = mybir.AluOpType
AF = mybir.ActivationFunctionType

# ---------------- geometry ----------------
B, H, W = 4, 352, 1216
HALF = 176               # output rows per core
NC = 8
C0 = HALF + 8            # 184: rows where fields/iter-0 feat are computed
GR = C0 + 2              # 186: guidance rows needed (conv halo)
FR = C0 + 4              # 188: feat rows (init + buffer)
WG = W + 2               # 1218: guidance cols incl conv pad
WF = W + 4               # 1220: feat cols incl +-2 pad
CH = 8                   # conv row-chunk
NCHUNK = C0 // CH        # 23
NT = 19                  # 512-px tiles per chunk (8 rows x 64 cols)
HW2 = W // 2             # 608 col half
FS = C0 * W              # field plane stride (184*1216)

# conv output channel order (M = 94):
#  offsets occupy m 0..31 and 35..56 (pos = 18*k + idx; m = pos if pos<32
#  else pos+3); m 32..34 are conv-pad rows overwritten by the S matmul
#  (PSUM matmul-out base must be 0/32/64); m 64..93: logits (exp reads at
#  base 64); m 57..63 pad.
MM = 94

_CACHE = {}


def _dap(t, offset, dims):
    return bass.AP(tensor=t, offset=offset, ap=[list(d) for d in dims])


def _build_program(do_p1=True, do_p2=True, ntaps=9):
    nc = bacc.Bacc("TRN2", target_bir_lowering=False, debug=False)

    g_d = nc.dram_tensor("g", [30, GR, WG], BF16, kind="ExternalInput")
    w3_d = nc.dram_tensor("w3", [90, 3, MM], BF16, kind="ExternalInput")
    ones_d = nc.dram_tensor("ones_s", [30, 3], BF16, kind="ExternalInput")
    b94_d = nc.dram_tensor("b94", [MM, 1], F32, kind="ExternalInput")
    blog_d = nc.dram_tensor("blog", [30, 1], F32, kind="ExternalInput")
    fin_d = nc.dram_tensor("finit", [FR, WF], BF16, kind="ExternalInput")
    omc_d = nc.dram_tensor("omc", [C0, W], F16, kind="ExternalInput")
    cff_d = nc.dram_tensor("cff", [C0, W], F16, kind="ExternalInput")
    out_d = nc.dram_tensor("out", [HALF, W], F32, kind="ExternalOutput")

    featbuf_a = nc.dram_tensor("featbuf_a", [FR, WF], BF16, kind="Internal")
    featbuf_b = nc.dram_tensor("featbuf_b", [FR, WF], BF16, kind="Internal")
    offs_d = nc.dram_tensor("offs", [3, 18, C0, W], BF16, kind="Internal")
    es_d = nc.dram_tensor("es", [3, 10, C0, W], BF16, kind="Internal")
    ss_d = nc.dram_tensor("ss", [3, C0, W], BF16, kind="Internal")

    with tile.TileContext(nc) as tc, ExitStack() as octx:
        # ---- persistent small tiles ----
        singles = octx.enter_context(tc.tile_pool(name="singles", bufs=1))
        w3_sb = singles.tile([90, 3, MM], BF16, tag="w3")
        nc.sync.dma_start(out=w3_sb, in_=w3_d.ap())
        ones_sb = singles.tile([30, 3], BF16, tag="ones")
        nc.sync.dma_start(out=ones_sb, in_=ones_d.ap())
        b94_sb = singles.tile([MM, 1], F32, tag="b94")
        nc.sync.dma_start(out=b94_sb, in_=b94_d.ap())
        blog_sb = singles.tile([30, 1], F32, tag="blog")
        nc.sync.dma_start(out=blog_sb, in_=blog_d.ap())
        zt = singles.tile([1, 2 * FR], BF16, tag="zt")
        nc.vector.memset(zt, 0.0)
        # zero the feat-buffer column pads (rows never write cols [0,2)+[1218,1220))
        for fb in (featbuf_a, featbuf_b):
            nc.sync.dma_start(out=_dap(fb, 0, [[WF, FR], [1, 2]]),
                              in_=zt[:, 0:2 * FR])
            nc.sync.dma_start(out=_dap(fb, W + 2, [[WF, FR], [1, 2]]),
                              in_=zt[:, 0:2 * FR])

        # ================= Phase 1: conv + field extraction =================
        with ExitStack() as ctx:
            g3p = ctx.enter_context(tc.tile_pool(name="g3", bufs=2))
            stp = ctx.enter_context(tc.tile_pool(name="stage", bufs=2))
            pp = ctx.enter_context(tc.tile_pool(name="psA", bufs=4, space="PSUM"))

            for ci in range(NCHUNK if do_p1 else 0):
                g3 = g3p.tile([90, CH + 2, W], BF16, tag="g3")
                for kx in range(3):
                    nc.sync.dma_start(
                        out=g3[30 * kx:30 * kx + 30],
                        in_=_dap(g_d, (ci * CH) * WG + kx,
                                 [[GR * WG, 30], [WG, CH + 2], [1, W]]))
                all_st = stp.tile([57, CH, NT * 64], BF16, tag="all_st")
                e_st = stp.tile([30, CH, NT * 64], BF16, tag="e_st")
                pas = {}

                def s_and_evac(ti):
                    pa = pas.pop(ti)
                    # S_k = sum of iteration-k exps -> rows 32..34 of the same bank
                    nc.tensor.matmul(pa[32:35], ones_sb,
                                     e_st[:, :, ti * 64:(ti + 1) * 64],
                                     start=True, stop=True, skip_group_check=True)
                    # one evacuation: offsets(+bias) rows 0..31+35..56, S rows 32..34
                    if ti % 2 == 0:
                        nc.vector.tensor_scalar(
                            out=all_st[:, :, ti * 64:(ti + 1) * 64], in0=pa[0:57],
                            scalar1=b94_sb[0:57], scalar2=None, op0=ALU.add)
                    else:
                        nc.scalar.activation(
                            out=all_st[:, :, ti * 64:(ti + 1) * 64], in_=pa[0:57],
                            func=AF.Identity, bias=b94_sb[0:57], scale=1.0)

                for ti in range(NT):
                    pa = pp.tile([MM, 512], F32, tag="pa")
                    pas[ti] = pa
                    for ky in range(3):
                        nc.tensor.matmul(
                            pa[0:MM], w3_sb[:, ky],
                            g3[:, ky:ky + CH, ti * 64:(ti + 1) * 64],
                            start=(ky == 0), stop=(ky == 2))
                    nc.scalar.activation(
                        out=e_st[:, :, ti * 64:(ti + 1) * 64], in_=pa[64:94], func=AF.Exp,
                        bias=blog_sb, scale=1.0)
                    if ti > 0:
                        s_and_evac(ti - 1)
                s_and_evac(NT - 1)
                # spill chunk fields to DRAM
                ro = ci * CH * W
                nc.sync.dma_start(
                    out=_dap(offs_d, ro, [[FS, 32], [W, CH], [1, W]]), in_=all_st[0:32])
                nc.sync.dma_start(
                    out=_dap(offs_d, 32 * FS + ro, [[FS, 22], [W, CH], [1, W]]),
                    in_=all_st[35:57])
                nc.sync.dma_start(
                    out=_dap(es_d, ro, [[FS, 30], [W, CH], [1, W]]), in_=e_st)
                nc.sync.dma_start(
                    out=_dap(ss_d, ro, [[FS, 3], [W, CH], [1, W]]), in_=all_st[32:35])

        # ================= Phase 2: deformable iterations =================
        with ExitStack() as ctx:
            slabp = ctx.enter_context(tc.tile_pool(name="slab", bufs=2))
            dpp = ctx.enter_context(tc.tile_pool(name="dp", bufs=1))
            fldp = ctx.enter_context(tc.tile_pool(name="fld", bufs=1))
            relup = ctx.enter_context(tc.tile_pool(name="relu", bufs=1))
            gp = ctx.enter_context(tc.tile_pool(name="g", bufs=2))
            accp = ctx.enter_context(tc.tile_pool(name="acc", bufs=2))
            scrp = ctx.enter_context(tc.tile_pool(name="scr", bufs=2))

            for k in range(3 if do_p2 else 0):
                rk = C0 - 4 * k
                src_d = fin_d if k == 0 else (featbuf_a if k == 1 else featbuf_b)
                dst_fb = featbuf_a if k == 0 else featbuf_b
                for (lo, rows) in ((0, 64), (64, 64), (128, rk - 128)):
                    P = 2 * rows
                    ro = (2 * k + lo) * W          # field row offset
                    slab = slabp.tile([128, 5, 612], BF16, tag="slab")
                    for h in range(2):
                        nc.sync.dma_start(
                            out=slab[h * rows:(h + 1) * rows],
                            in_=_dap(src_d, (lo + 2 * k) * WF + HW2 * h,
                                     [[WF, rows], [WF, 5], [1, 612]]))
                    # slab1[c] = slab[c+1]: 4B-aligned mirror for odd column shifts
                    slab1 = slabp.tile([128, 5, 612], BF16, tag="slab1")
                    nc.sync.dma_start(out=slab1[0:P, :, 0:611], in_=slab[0:P, :, 1:612])
                    # dpa[c] = Dp[c] = slab[c+1]-slab[c]; dpb[c] = Dp[c+1]
                    dpa = dpp.tile([128, 5, 612], BF16, tag="dpa")
                    dpb = dpp.tile([128, 5, 612], BF16, tag="dpb")
                    nc.vector.tensor_tensor(out=dpa[0:P, :, 0:611], in0=slab1[0:P, :, 0:611],
                                            in1=slab[0:P, :, 0:611], op=ALU.subtract)
                    nc.vector.tensor_tensor(out=dpb[0:P, :, 0:610], in0=slab[0:P, :, 2:612],
                                            in1=slab1[0:P, :, 0:610], op=ALU.subtract)
                    # second differences: d2a[c] = Dp[c]-Dp[c-1], d2b[c] = Dp[c+1]-Dp[c]
                    d2a = dpp.tile([128, 5, 612], BF16, tag="d2a")
                    d2b = dpp.tile([128, 5, 612], BF16, tag="d2b")
                    nc.vector.tensor_tensor(out=d2a[0:P, :, 2:611], in0=dpa[0:P, :, 2:611],
                                            in1=dpb[0:P, :, 0:609], op=ALU.subtract)
                    nc.vector.tensor_tensor(out=d2b[0:P, :, 0:610], in0=dpb[0:P, :, 0:610],
                                            in1=dpa[0:P, :, 0:610], op=ALU.subtract)

                    def sl_view(a, off, n=HW2):
                        t_, o_ = (slab, off) if off % 2 == 0 else (slab1, off - 1)
                        return t_[0:P, 2 + a, o_:o_ + n]

                    def dp_view(a, off, n=HW2):
                        t_, o_ = (dpa, off) if off % 2 == 0 else (dpb, off - 1)
                        return t_[0:P, 2 + a, o_:o_ + n]

                    def d2p_view(a, off, n=HW2):
                        t_, o_ = (d2a, off) if off % 2 == 0 else (d2b, off - 1)
                        return t_[0:P, 2 + a, o_:o_ + n]

                    def ldfield(dst, base_d, off0, nf=1):
                        for h in range(2):
                            dims = [[W, rows]] + ([[FS, nf]] if nf > 1 else []) + [[1, HW2]]
                            nc.sync.dma_start(
                                out=dst[h * rows:(h + 1) * rows],
                                in_=_dap(base_d, off0 + ro + HW2 * h, dims))

                    omc_t = scrp.tile([128, HW2], F16, tag="omc")
                    ldfield(omc_t, omc_d, 0)
                    cff_t = scrp.tile([128, HW2], F16, tag="cff")
                    ldfield(cff_t, cff_d, 0)
                    s_t = scrp.tile([128, HW2], BF16, tag="s_t")
                    ldfield(s_t, ss_d, k * FS)
                    rs_t = scrp.tile([128, HW2], F32, tag="rs")
                    nc.vector.reciprocal(out=rs_t[0:P], in_=s_t[0:P])
                    omcrs = scrp.tile([128, HW2], F32, tag="omcrs")
                    nc.vector.tensor_tensor(out=omcrs[0:P], in0=omc_t[0:P],
                                            in1=rs_t[0:P], op=ALU.mult)

                    offt = fldp.tile([128, 18, HW2], BF16, tag="offt")
                    ldfield(offt, offs_d, k * 18 * FS, nf=18)
                    est = fldp.tile([128, 10, HW2], BF16, tag="est")
                    ldfield(est, es_d, k * 10 * FS, nf=10)
                    prop = accp.tile([128, HW2], F32, tag="prop")
                    for t in range(ntaps):
                        ky, kx = t // 3, t % 3
                        bb = kx - 1
                        e_t = est[:, t]
                        up = relup.tile([128, HW2], BF16, tag="up")
                        vp = relup.tile([128, HW2], BF16, tag="vp")
                        vm = relup.tile([128, HW2], BF16, tag="vm")
                        nc.scalar.activation(out=up[0:P], in_=offt[0:P, 2 * t + 1], func=AF.Relu)
                        nc.scalar.activation(out=vp[0:P], in_=offt[0:P, 2 * t], func=AF.Relu)
                        nc.scalar.activation(out=vm[0:P], in_=offt[0:P, 2 * t], func=AF.Relu, scale=-1.0)

                        dxv = offt[:, 2 * t + 1]
                        Gs = {}
                        for rho, eng in ((-1, nc.vector), (0, nc.vector), (1, nc.vector)):
                            a = ky - 1 + rho
                            sl = sl_view(a, 2 + bb)
                            dd2 = d2p_view(a, 2 + bb)
                            d2v = dp_view(a, 1 + bb)
                            t1 = scrp.tile([128, HW2], BF16, tag=f"xs{rho}")
                            t2 = scrp.tile([128, HW2], BF16, tag=f"xt{rho}")
                            G = gp.tile([128, HW2], BF16, tag=f"G{rho}")
                            eng.tensor_tensor(out=t1[0:P], in0=up[0:P], in1=dd2, op=ALU.mult)
                            eng.tensor_tensor(out=t2[0:P], in0=dxv[0:P], in1=d2v, op=ALU.mult)
                            eng.tensor_tensor(out=t1[0:P], in0=t1[0:P], in1=t2[0:P], op=ALU.add)
                            eng.tensor_tensor(out=G[0:P], in0=sl, in1=t1[0:P], op=ALU.add)
                            Gs[rho] = G
                        d1 = scrp.tile([128, HW2], BF16, tag="d1")
                        nc.vector.tensor_tensor(out=d1[0:P], in0=Gs[1][0:P], in1=Gs[0][0:P], op=ALU.subtract)
                        nc.vector.tensor_tensor(out=d1[0:P], in0=vp[0:P], in1=d1[0:P], op=ALU.mult)
                        d2 = scrp.tile([128, HW2], BF16, tag="d2")
                        nc.gpsimd.tensor_tensor(out=d2[0:P], in0=Gs[-1][0:P], in1=Gs[0][0:P], op=ALU.subtract)
                        nc.gpsimd.tensor_tensor(out=d2[0:P], in0=vm[0:P], in1=d2[0:P], op=ALU.mult)
                        val = scrp.tile([128, HW2], BF16, tag="val")
                        nc.vector.tensor_tensor(out=val[0:P], in0=d1[0:P], in1=d2[0:P], op=ALU.add)
                        nc.vector.tensor_tensor(out=val[0:P], in0=Gs[0][0:P], in1=val[0:P], op=ALU.add)
                        if t == 0:
                            nc.vector.tensor_tensor(out=prop[0:P], in0=e_t[0:P], in1=val[0:P], op=ALU.mult)
                        else:
                            tv = scrp.tile([128, HW2], F32, tag="tv")
                            nc.vector.tensor_tensor(out=tv[0:P], in0=e_t[0:P], in1=val[0:P], op=ALU.mult)
                            nc.vector.tensor_tensor(out=prop[0:P], in0=prop[0:P], in1=tv[0:P], op=ALU.add)

                    tfe = scrp.tile([128, HW2], F32, tag="tfe")
                    nc.gpsimd.tensor_tensor(out=tfe[0:P], in0=est[0:P, 9],
                                            in1=sl_view(0, 2), op=ALU.mult)
                    nc.vector.tensor_tensor(out=prop[0:P], in0=prop[0:P], in1=tfe[0:P], op=ALU.add)
                    nc.vector.tensor_tensor(out=prop[0:P], in0=prop[0:P], in1=omcrs[0:P], op=ALU.mult)
                    fnew = accp.tile([128, HW2], F32 if k == 2 else BF16,
                                     tag="fnew32" if k == 2 else "fnew16")
                    nc.vector.tensor_tensor(out=fnew[0:P], in0=prop[0:P], in1=cff_t[0:P], op=ALU.add)
                    for h in range(2):
                        if k < 2:
                            dst = _dap(dst_fb, (2 + 2 * k + lo) * WF + 2 + HW2 * h,
                                       [[WF, rows], [1, HW2]])
                        else:
                            dst = _dap(out_d, lo * W + HW2 * h, [[W, rows], [1, HW2]])
                        nc.sync.dma_start(out=dst, in_=fnew[h * rows:(h + 1) * rows])

    nc.compile()
    return nc


def _prep_inputs(inputs):
    """Full inputs -> list of 8 per-core input dicts (host-side shard+pad)."""
    feat_init = np.asarray(inputs["feat_init"], np.float32)
    guidance = np.asarray(inputs["guidance"], np.float32)
    confidence = np.asarray(inputs["confidence"], np.float32)
    feat_fix = np.asarray(inputs["feat_fix"], np.float32)
    W_conv = np.asarray(inputs["W_conv"], np.float32)
    b_conv = np.asarray(inputs["b_conv"], np.float32)

    # channel reorder: original channel o -> (k = o//28, idx = o%28)
    perm_m = np.zeros(84, np.int64)
    bias94 = np.zeros((MM, 1), np.float32)
    for o in range(84):
        k, idx = o // 28, o % 28
        if idx < 18:
            pos = 18 * k + idx
            m = pos if pos < 32 else pos + 3
        else:
            m = 64 + 10 * k + (idx - 18)
        perm_m[o] = m
        bias94[m, 0] = b_conv[o]
    w3 = np.zeros((90, 3, MM), np.float32)
    for o in range(84):
        for c in range(30):
            for ky in range(3):
                for kx in range(3):
                    w3[kx * 30 + c, ky, perm_m[o]] = W_conv[o, c, ky, kx]
    w3 = w3.astype(ml_dtypes.bfloat16)
    ones_s = np.zeros((30, 3), ml_dtypes.bfloat16)
    for k in range(3):
        ones_s[10 * k:10 * k + 10, k] = 1.0

    conf = np.sign(feat_fix) * (1.0 / (1.0 + np.exp(-confidence)))
    omc_full = (1.0 - conf)[:, 0].astype(np.float32)     # [B,H,W]
    cff_full = (conf * feat_fix)[:, 0].astype(np.float32)

    def pad_rows(img, lo, hi, fill=0.0):
        """rows [lo, hi) of img [H, ...] with zero padding outside."""
        out = np.full((hi - lo,) + img.shape[1:], fill, img.dtype)
        s0, s1 = max(lo, 0), min(hi, H)
        out[s0 - lo:s1 - lo] = img[s0:s1]
        return out

    in_maps = []
    for core in range(NC):
        b, half = core // 2, core % 2
        r0 = half * HALF
        g_sh = np.zeros((30, GR, WG), np.float32)
        glo, ghi = r0 - 5, r0 + HALF + 5
        s0, s1 = max(glo, 0), min(ghi, H)
        g_sh[:, s0 - glo:s1 - glo, 1:W + 1] = guidance[b, :, s0:s1, :]
        f_sh = np.zeros((FR, WF), np.float32)
        flo, fhi = r0 - 6, r0 + HALF + 6
        s0, s1 = max(flo, 0), min(fhi, H)
        f_sh[s0 - flo:s1 - flo, 2:W + 2] = feat_init[b, 0, s0:s1, :]
        in_maps.append({
            "g": g_sh.astype(ml_dtypes.bfloat16),
            "w3": w3,
            "ones_s": ones_s,
            "b94": bias94,
            "blog": np.ascontiguousarray(bias94[64:94]),
            "finit": f_sh.astype(ml_dtypes.bfloat16),
            "omc": np.ascontiguousarray(pad_rows(omc_full[b], r0 - 4, r0 + HALF + 4)).astype(np.float16),
            "cff": np.ascontiguousarray(pad_rows(cff_full[b], r0 - 4, r0 + HALF + 4)).astype(np.float16),
        })
    return in_maps


def kernel(**inputs) -> np.ndarray:
    if "nc" not in _CACHE:
        _CACHE["nc"] = _build_program()
    nc = _CACHE["nc"]
    in_maps = _prep_inputs(inputs)
    trace = os.environ.get("KERNEL_TRACE", "0") == "1"
    res = run_bass_kernel_spmd(nc, in_maps, core_ids=list(range(NC)), trace=trace)
    _CACHE["last_result"] = res
    out = np.zeros((B, 1, H, W), np.float32)
    for core in range(NC):
        b, half = core // 2, core % 2
        out[b, 0, half * HALF:(half + 1) * HALF, :] = res.results[core]["out"]
    return out
